# revision 36
# baseline (speedup 1.0000x reference)
"""Trainium2 Bass kernel for causal multi-head attention with RoPE.

Problem: x[2,2048,2048], 16 heads, head_dim 128, fp32.
  q/k/v = x @ w{q,k,v}^T ; RoPE on q,k ; causal softmax(q k^T / sqrt(128)) @ v ; out @ wo^T

Sharding: Megatron tensor-parallel over heads — 2 heads per core on 8 cores.
Each core computes a partial y (its 2 heads' contribution through wo); the host
sums the 8 partials.  No device collectives.

Per-core layout (all matmuls bf16 with f32 PSUM accumulation):
  - xT bf16, host pre-transposed + tiled [tt, cpair, 128, 1024] (2KB DMA
    lines), every DMA partition-split in two and issued from two engine
    queues (sync + gpsimd) for queue parallelism
  - weights host-prepacked in SBUF layout (contiguous 2KB lines per DMA part)
  - q^T, k^T computed feature-major [head_dim, tokens]; v token-major [tok, d]
  - scores transposed: S^T[key, q] = kT.T @ qT; on diagonal-crossing key tiles
    only columns q >= key-block start are computed (partial moving free dim),
    with one [128,128] triangular mask at the diagonal block itself
  - softmax without max subtraction (scores bounded, exp safe in f32):
      P^T = exp(S^T/sqrt(128)) on ACT, row-sums by DVE-accumulating P^T tiles
      then ONE ones-column matmul per (b,qt,h), 1/r via reciprocal_approx_fast,
      o^T = v.T @ P^T in PSUM, normalized by broadcast multiply
  - y rows = (o_norm^T).T @ woT written token-major bf16; host sums in f32
  - schedule: iteration tt emits token tile tt (dense PE burst at max PE
    p-state), then attention unit (b,qt)=divmod(tt,QT) with the PREVIOUS
    unit's output projection dripped into the kt loop as PE filler (the
    exp->AV chain otherwise starves the PE); a few filler steps are reserved
    for the tile->attention seam where score(0) waits on the fresh q RoPE.
"""

import math
import sys

sys.path.insert(0, "/opt/trn_rl_repo")

import ml_dtypes  # noqa: E402
import numpy as np  # noqa: E402

P = 128
D = 2048
HD = 128  # head dim
B = 2
T = 2048
TOK = B * T  # 4096
NCORES = 8
HPC = 2  # heads per core
DC = HPC * HD  # 256 dims per core
CCHUNKS = D // P  # 16 contraction chunks
CPAIRS = CCHUNKS // 2  # 8 chunk pairs per x tile DMA
TT = TOK // 512  # 8 token tiles of 512
QT = T // 512  # 4 query tiles per batch
KT_PER_Q = 512 // P  # 4 key tiles per query tile
SEAM_STEPS = 5  # yproj filler steps run before the unit's first score

_CACHE = {}


def _build_nc():
    import concourse.bacc as bacc
    import concourse.mybir as mybir
    import concourse.tile as tile

    f32 = mybir.dt.float32
    bf16 = mybir.dt.bfloat16

    nc = bacc.Bacc("TRN2", target_bir_lowering=False, debug=False, num_devices=NCORES)

    xTt = nc.dram_tensor("xTt", [TT, CPAIRS, P, 1024], bf16,
                         kind="ExternalInput").ap()
    cosT = nc.dram_tensor("cosT", [HD, TOK], bf16, kind="ExternalInput").ap()
    sinT = nc.dram_tensor("sinT", [HD, TOK], bf16, kind="ExternalInput").ap()
    # weights already in SBUF layout: [ci, chunk, dc] / [ki, h, n]
    wqP = nc.dram_tensor("wqP", [P, CCHUNKS, DC], bf16, kind="ExternalInput").ap()
    wkP = nc.dram_tensor("wkP", [P, CCHUNKS, DC], bf16, kind="ExternalInput").ap()
    wvP = nc.dram_tensor("wvP", [P, CCHUNKS, DC], bf16, kind="ExternalInput").ap()
    woP = nc.dram_tensor("woP", [P, HPC, D], bf16, kind="ExternalInput").ap()
    y = nc.dram_tensor("y", [TOK, D], bf16, kind="ExternalOutput").ap()

    inv_sqrt_hd = 1.0 / math.sqrt(HD)

    with tile.TileContext(nc) as tc:
        with (
            tc.tile_pool(name="consts", bufs=1) as consts,
            tc.tile_pool(name="wpool", bufs=1) as wpool,
            tc.tile_pool(name="qkv", bufs=1) as qkv,
            tc.tile_pool(name="xp", bufs=4) as xp,
            tc.tile_pool(name="csp", bufs=2) as csp,
            tc.tile_pool(name="ropep", bufs=1) as ropep,
            tc.tile_pool(name="ptp", bufs=8) as ptp,
            tc.tile_pool(name="pap", bufs=2) as pap,
            tc.tile_pool(name="rrp", bufs=2) as rrp,
            tc.tile_pool(name="bcp", bufs=2) as bcp,
            tc.tile_pool(name="onp", bufs=3) as onp,
            tc.tile_pool(name="ysp", bufs=3) as ysp,
            tc.tile_pool(name="ps", bufs=8, space="PSUM") as ps,
        ):
            # ---- constants ----
            # single [128,128] triangular causal mask: keep q_local >= key_local
            mask128 = consts.tile([P, P], bf16, tag="mask128")
            nc.gpsimd.memset(mask128[:], 1.0)
            nc.gpsimd.affine_select(
                out=mask128[:], in_=mask128[:], compare_op=mybir.AluOpType.is_ge,
                fill=0.0, base=0, channel_multiplier=-1, pattern=[[1, P]],
            )
            ones_col = consts.tile([P, 1], bf16, tag="ones_col")
            nc.gpsimd.memset(ones_col[:], 1.0)

            # ---- resident weights ----
            wq_t = wpool.tile([P, CCHUNKS, DC], bf16, tag="wq")
            wk_t = wpool.tile([P, CCHUNKS, DC], bf16, tag="wk")
            wv_t = wpool.tile([P, CCHUNKS, DC], bf16, tag="wv")
            wo_t = wpool.tile([P, HPC, D], bf16, tag="wo")

            # chunk ranges per x-pair slot: small first part so chunk-0
            # matmuls are not stuck behind 768KB of weight DMA at cold start
            W_PARTS = [(0, 2), (2, 6), (6, 10), (10, 14), (14, 16)]

            def emit_w_part(j):
                csl = slice(*W_PARTS[j])
                for wt, wdram in ((wq_t, wqP), (wk_t, wkP), (wv_t, wvP)):
                    nc.scalar.dma_start(wt[:, csl, :], wdram[:, csl, :])

            # ---- resident activations ----
            qT_t = qkv.tile([P, HPC, TOK], bf16, tag="qT")  # [head_dim, h, tok]
            kT_t = qkv.tile([P, HPC, TOK], bf16, tag="kT")
            v_t = qkv.tile([P, TOK // P, DC], bf16, tag="v")  # [tok%128, tokblk, d]

            # ---- token-tile body: q/k/v projections + RoPE ----
            def emit_tile(tt, hook=None):
                tsl = slice(tt * 512, (tt + 1) * 512)
                cos_t = csp.tile([P, 512], bf16, tag="cos")
                nc.scalar.dma_start(cos_t[:], cosT[:, tsl])
                sin_t = csp.tile([P, 512], bf16, tag="sin")
                nc.scalar.dma_start(sin_t[:], sinT[:, tsl])

                pq = [ps.tile([P, 512], f32, tag="ps", name=f"pq{i}") for i in range(HPC)]
                pk = [ps.tile([P, 512], f32, tag="ps", name=f"pk{i}") for i in range(HPC)]
                # two banks hold all four v accumulators ([t128, 256] pairs packed
                # side by side).  Only the first half's c==0 matmul uses start=True
                # (clears the whole bank); the second half's first matmul then
                # overwrites its still-clean elements via has_written bits.
                pv = [ps.tile([P, 512], f32, tag="ps", name=f"pv{i}") for i in range(2)]

                for cp in range(CPAIRS):
                    if cp == 2 and hook is not None:
                        hook()
                    xt = xp.tile([P, 1024], bf16, tag="x")
                    # partition-split halves: two hardware DMA queues per pair
                    nc.sync.dma_start(xt[0:64, :], xTt[tt, cp, 0:64, :])
                    nc.sync.dma_start(xt[64:128, :], xTt[tt, cp, 64:128, :])
                    if tt == 0 and cp < len(W_PARTS):
                        emit_w_part(cp)
                    if tt == 1 and cp < 4:
                        # wo in 4 parts (2KB lines); first needed by the
                        # yproj filler dripped into attention unit (0,1)
                        h, nh = divmod(cp, 2)
                        nc.scalar.dma_start(
                            wo_t[:, h, nh * 1024:(nh + 1) * 1024],
                            woP[:, h, nh * 1024:(nh + 1) * 1024])
                    for ci in range(2):
                        c = 2 * cp + ci
                        xtr = xt[:, ci * 512:(ci + 1) * 512]
                        st, sp = (c == 0), (c == CCHUNKS - 1)
                        for h in range(HPC):
                            dsl = slice(h * HD, (h + 1) * HD)
                            nc.tensor.matmul(pq[h][:], wq_t[:, c, dsl], xtr,
                                             start=st, stop=sp)
                            nc.tensor.matmul(pk[h][:], wk_t[:, c, dsl], xtr,
                                             start=st, stop=sp)
                        for s4 in range(4):
                            half = s4 % 2
                            nc.tensor.matmul(pv[s4 // 2][:, half * DC:(half + 1) * DC],
                                             xt[:, ci * 512 + s4 * P:ci * 512 + (s4 + 1) * P],
                                             wv_t[:, c, :],
                                             start=st and half == 0, stop=sp,
                                             skip_group_check=half == 1)

                # Evacuate PSUM: q copies (ACT) feed the q RoPE immediately —
                # the next attention unit's first scores wait on it.  k copies
                # (DVE) and k RoPE after the q chain; v copies on ACT.
                for h in range(HPC):
                    nc.scalar.copy(qT_t[:, h, tsl], pq[h][:])

                def rope(dst_t, h):
                    dst = dst_t[:, h, tsl]
                    rot = ropep.tile([P, 512], bf16, tag="rot")
                    nc.vector.tensor_scalar_mul(rot[0:64, :], dst[64:128, :], -1.0)
                    nc.vector.tensor_copy(rot[64:128, :], dst[0:64, :])
                    nc.vector.tensor_mul(out=rot[:], in0=rot[:], in1=sin_t[:])
                    nc.vector.tensor_mul(out=dst, in0=dst, in1=cos_t[:])
                    nc.vector.tensor_add(out=dst, in0=dst, in1=rot[:])

                rope(qT_t, 0)
                nc.vector.tensor_copy(kT_t[:, 0, tsl], pk[0][:])
                rope(kT_t, 0)
                rope(qT_t, 1)
                nc.vector.tensor_copy(kT_t[:, 1, tsl], pk[1][:])
                rope(kT_t, 1)
                for s4 in range(4):
                    half = s4 % 2
                    nc.scalar.copy(v_t[:, tt * 4 + s4, :],
                                   pv[s4 // 2][:, half * DC:(half + 1) * DC])

            # ---- output projection of a finished unit, as a list of 16 small
            # steps (one PSUM bank: 2 matmuls + evacuation) for dripping into
            # the next unit's attention as PE filler ----
            def make_yproj_steps(onorm, b, qt, all_dve):
                ystages = {}

                def step(s4, dout):
                    if dout == 0:
                        ystages[s4] = ysp.tile([P, D], bf16, tag="ystage",
                                               name="ystage")
                    ystage = ystages[s4]
                    py = ps.tile([P, 512], f32, tag="ps", name="py")
                    for h in range(HPC):
                        nc.tensor.matmul(
                            py[:],
                            onorm[:, h, s4 * P:(s4 + 1) * P],
                            wo_t[:, h, dout * 512:(dout + 1) * 512],
                            start=(h == 0), stop=(h == HPC - 1))
                    # PSUM evacuation: while attention runs, ACT must stay
                    # a pure exp stream (a copy between exps stalls the AV
                    # chain), so filler copies go to DVE; the epilogue drain
                    # (no exps pending) alternates.
                    if not all_dve and (s4 * 4 + dout) % 2 == 0:
                        nc.scalar.copy(ystage[:, dout * 512:(dout + 1) * 512], py[:])
                    else:
                        nc.vector.tensor_copy(ystage[:, dout * 512:(dout + 1) * 512], py[:])
                    if dout == 3:
                        r0 = b * T + qt * 512 + s4 * P
                        nc.sync.dma_start(y[r0:r0 + P, :], ystage[:])

                return [(step, s4, dout) for s4 in range(4) for dout in range(4)]

            pending = []
            norm_queue = []

            def finish_norm():
                # row-sum matmuls + normalization of the previous unit,
                # emitted a few chunks into the next tile's burst: by then
                # the unit's DVE tail (last pacc adds) has long drained, so
                # the pr matmuls slot into the burst without a PE wait.
                if not norm_queue:
                    return
                paccs, onorm, b, qt = norm_queue.pop(0)
                for h in range(HPC):
                    pr = ps.tile([P, 512], f32, tag="ps", name="pr")
                    nc.tensor.matmul(pr[0:1, :], ones_col[:], paccs[h][:],
                                     start=True, stop=True)
                    rr = rrp.tile([1, 512], f32, tag="rr")
                    nc.vector.reciprocal_approx_fast(rr[:], pr[0:1, :])
                    bc = bcp.tile([P, 512], f32, tag="bc")
                    nc.gpsimd.partition_broadcast(bc[:], rr[:])
                    nc.vector.tensor_mul(out=onorm[:, h, :],
                                         in0=onorm[:, h, :], in1=bc[:])

            # ---- attention for one (batch, query-tile) unit ----
            def emit_attn(b, qt, filler):
                nkt = KT_PER_Q * (qt + 1)
                fstate = [0.0, 0]  # fractional accumulator, next index

                def run_steps(n):
                    end = min(fstate[1] + n, len(filler))
                    while fstate[1] < end:
                        f, s4, dout = filler[fstate[1]]
                        f(s4, dout)
                        fstate[1] += 1

                # seam: PE filler while DVE finishes the fresh tile's RoPE.
                # qt==0 units attend ONLY the fresh tile (every score waits on
                # its k RoPE), so they take their entire filler upfront.
                run_steps(len(filler) if qt == 0 else SEAM_STEPS)
                per_slot = (len(filler) - fstate[1]) / max(1, HPC * nkt)

                def drip():
                    fstate[0] += per_slot
                    n = int(fstate[0])
                    if n:
                        fstate[0] -= n
                        run_steps(n)

                q0 = b * T + qt * 512
                onorm = onp.tile([P, HPC, 512], bf16, tag="onorm")
                paccs = []
                for h in range(HPC):
                    po = ps.tile([P, 512], f32, tag="ps")
                    pacc = pap.tile([P, 512], bf16, tag="pacc")

                    def emit_score(kt, b=b, qt=qt, h=h, q0=q0):
                        # diagonal-crossing tiles only need columns q >= kt*128
                        off = max(0, (kt - KT_PER_Q * qt) * P)
                        ksl = slice(b * T + kt * P, b * T + (kt + 1) * P)
                        pscore = ps.tile([P, 512], f32, tag="ps", name="pscore")
                        nc.tensor.matmul(pscore[:, off:], kT_t[:, h, ksl],
                                         qT_t[:, h, q0 + off:q0 + 512],
                                         start=True, stop=True)
                        ptile = ptp.tile([P, 512], bf16, tag="pt", name="ptile")
                        nc.scalar.activation(ptile[:, off:], pscore[:, off:],
                                             mybir.ActivationFunctionType.Exp,
                                             scale=inv_sqrt_hd)
                        if kt >= KT_PER_Q * qt:
                            # triangular mask on the single diagonal block
                            nc.vector.tensor_mul(out=ptile[:, off:off + P],
                                                 in0=ptile[:, off:off + P],
                                                 in1=mask128[:])
                        return ptile, off

                    # kt loop pipelined by two: scores for kt+1/kt+2 are
                    # issued before the exp-gated AV matmul of kt, and filler
                    # runs between them, so the PE has wait-free work during
                    # exp and the exp chain gets a two-matmul head start.
                    ptiles = {0: emit_score(0)}
                    if nkt > 1:
                        ptiles[1] = emit_score(1)
                    for kt in range(nkt):
                        if kt + 2 < nkt:
                            ptiles[kt + 2] = emit_score(kt + 2)
                        drip()
                        ptile, off = ptiles.pop(kt)
                        st, sp = (kt == 0), (kt == nkt - 1)
                        nc.tensor.matmul(po[:, off:], v_t[:, b * (T // P) + kt,
                                                          h * HD:(h + 1) * HD],
                                         ptile[:, off:], start=st, stop=sp,
                                         skip_group_check=off > 0)
                        # row-sum accumulation on DVE (all-bf16 2x mode)
                        if kt == 0:
                            nc.vector.tensor_copy(pacc[:], ptile[:])
                        else:
                            nc.vector.tensor_add(out=pacc[:, off:],
                                                 in0=pacc[:, off:],
                                                 in1=ptile[:, off:])
                    # copy o out of PSUM right away (frees the bank); the
                    # row-sum matmul + normalization are deferred into the
                    # NEXT tile's burst so the PE never waits on the last
                    # pacc add (DVE) — a wait there would also stall the next
                    # tile queued behind it.
                    if qt >= 2:
                        nc.vector.tensor_copy(onorm[:, h, :], po[:])
                    else:
                        nc.scalar.copy(onorm[:, h, :], po[:])
                    paccs.append(pacc)

                # leftover filler (short units have more filler than slots)
                run_steps(len(filler))
                norm_queue.append((paccs, onorm, b, qt))
                pending.append((onorm, b, qt))

            # ---- schedule: iteration tt = dense tile burst, then attention
            # unit (b,qt)=divmod(tt,QT) with previous unit's yproj dripped in.
            # Tile-before-its-unit keeps the ACT queue flowing (tile PSUM
            # copies run at burst end, right before the unit's exps); the
            # fresh-RoPE wait of qt==0 units is covered by taking their
            # entire filler upfront. ----
            for tt in range(TT):
                b, qt = divmod(tt, QT)
                emit_tile(tt, hook=finish_norm)
                filler = (make_yproj_steps(*pending.pop(0), all_dve=True)
                          if pending else [])
                emit_attn(b, qt, filler)
            finish_norm()
            for args in pending:
                for f, s4, dout in make_yproj_steps(*args, all_dve=False):
                    f(s4, dout)

    nc.compile()
    return nc


def get_nc():
    if "nc" not in _CACHE:
        _CACHE["nc"] = _build_nc()
    return _CACHE["nc"]


def make_in_maps(x, cos, sin, wq, wk, wv, wo):
    bf16 = ml_dtypes.bfloat16
    xT = np.asarray(x, dtype=np.float32).reshape(TOK, D).T  # [D, TOK]
    # [D, TOK] -> [cp, ci(2), 128, tt, 512] -> [tt, cp, 128, ci, 512]
    xTt = np.ascontiguousarray(
        xT.reshape(CPAIRS, 2, P, TT, 512).transpose(3, 0, 2, 1, 4)
    ).reshape(TT, CPAIRS, P, 1024).astype(bf16)
    cosT = np.ascontiguousarray(
        np.asarray(cos, dtype=np.float32).reshape(TOK, HD).T).astype(bf16)
    sinT = np.ascontiguousarray(
        np.asarray(sin, dtype=np.float32).reshape(TOK, HD).T).astype(bf16)
    wq = np.asarray(wq, dtype=np.float32)
    wk = np.asarray(wk, dtype=np.float32)
    wv = np.asarray(wv, dtype=np.float32)
    wo = np.asarray(wo, dtype=np.float32)

    def packw(w, dsl):
        # w[dsl, :].T is [D, DC] with row d = co*128 + ci; SBUF wants [ci, co, dc]
        return np.ascontiguousarray(
            w[dsl, :].T.reshape(CCHUNKS, P, DC).transpose(1, 0, 2)).astype(bf16)

    in_maps = []
    for c in range(NCORES):
        dsl = slice(c * DC, (c + 1) * DC)
        # wo[:, dsl].T is [DC, D] with row k = ko*128 + ki; SBUF wants [ki, ko, n]
        woPm = np.ascontiguousarray(
            wo[:, dsl].T.reshape(HPC, P, D).transpose(1, 0, 2)).astype(bf16)
        in_maps.append({
            "xTt": xTt,
            "cosT": cosT,
            "sinT": sinT,
            "wqP": packw(wq, dsl),
            "wkP": packw(wk, dsl),
            "wvP": packw(wv, dsl),
            "woP": woPm,
        })
    return in_maps


def kernel(x, cos, sin, wq, wk, wv, wo):
    from concourse.bass_utils import run_bass_kernel_spmd

    nc = get_nc()
    in_maps = make_in_maps(x, cos, sin, wq, wk, wv, wo)
    res = run_bass_kernel_spmd(nc, in_maps, list(range(NCORES)))
    out = np.zeros((TOK, D), dtype=np.float32)
    for m in res.results:
        out += m["y"].astype(np.float32)
    return out.reshape(B, T, D)


# revision 37
# speedup vs baseline: 1.0027x; 1.0027x over previous
"""Trainium2 Bass kernel for causal multi-head attention with RoPE.

Problem: x[2,2048,2048], 16 heads, head_dim 128, fp32.
  q/k/v = x @ w{q,k,v}^T ; RoPE on q,k ; causal softmax(q k^T / sqrt(128)) @ v ; out @ wo^T

Sharding: Megatron tensor-parallel over heads — 2 heads per core on 8 cores.
Each core computes a partial y (its 2 heads' contribution through wo); the host
sums the 8 partials.  No device collectives.

Per-core layout (all matmuls bf16 with f32 PSUM accumulation):
  - xT bf16, host pre-transposed + tiled [tt, cpair, 128, 1024] (2KB DMA
    lines), every DMA partition-split in two and issued from two engine
    queues (sync + gpsimd) for queue parallelism
  - weights host-prepacked in SBUF layout (contiguous 2KB lines per DMA part)
  - q^T, k^T computed feature-major [head_dim, tokens]; v token-major [tok, d]
  - scores transposed: S^T[key, q] = kT.T @ qT; on diagonal-crossing key tiles
    only columns q >= key-block start are computed (partial moving free dim),
    with one [128,128] triangular mask at the diagonal block itself
  - softmax without max subtraction (scores bounded, exp safe in f32):
      P^T = exp(S^T/sqrt(128)) on ACT, row-sums by DVE-accumulating P^T tiles
      then ONE ones-column matmul per (b,qt,h), 1/r via reciprocal_approx_fast,
      o^T = v.T @ P^T in PSUM, normalized by broadcast multiply
  - y rows = (o_norm^T).T @ woT written token-major bf16; host sums in f32
  - schedule: iteration tt emits token tile tt (dense PE burst at max PE
    p-state), then attention unit (b,qt)=divmod(tt,QT) with the PREVIOUS
    unit's output projection dripped into the kt loop as PE filler (the
    exp->AV chain otherwise starves the PE); a few filler steps are reserved
    for the tile->attention seam where score(0) waits on the fresh q RoPE.
"""

import math
import sys

sys.path.insert(0, "/opt/trn_rl_repo")

import ml_dtypes  # noqa: E402
import numpy as np  # noqa: E402

P = 128
D = 2048
HD = 128  # head dim
B = 2
T = 2048
TOK = B * T  # 4096
NCORES = 8
HPC = 2  # heads per core
DC = HPC * HD  # 256 dims per core
CCHUNKS = D // P  # 16 contraction chunks
CPAIRS = CCHUNKS // 2  # 8 chunk pairs per x tile DMA
TT = TOK // 512  # 8 token tiles of 512
QT = T // 512  # 4 query tiles per batch
KT_PER_Q = 512 // P  # 4 key tiles per query tile
SEAM_STEPS = 5  # yproj filler steps run before the unit's first score

_CACHE = {}


def _build_nc():
    import concourse.bacc as bacc
    import concourse.mybir as mybir
    import concourse.tile as tile

    f32 = mybir.dt.float32
    bf16 = mybir.dt.bfloat16

    nc = bacc.Bacc("TRN2", target_bir_lowering=False, debug=False, num_devices=NCORES)

    xTt = nc.dram_tensor("xTt", [TT, CPAIRS, P, 1024], bf16,
                         kind="ExternalInput").ap()
    cosT = nc.dram_tensor("cosT", [HD, TOK], bf16, kind="ExternalInput").ap()
    sinT = nc.dram_tensor("sinT", [HD, TOK], bf16, kind="ExternalInput").ap()
    # weights already in SBUF layout: [ci, chunk, dc] / [ki, h, n]
    wqP = nc.dram_tensor("wqP", [P, CCHUNKS, DC], bf16, kind="ExternalInput").ap()
    wkP = nc.dram_tensor("wkP", [P, CCHUNKS, DC], bf16, kind="ExternalInput").ap()
    wvP = nc.dram_tensor("wvP", [P, CCHUNKS, DC], bf16, kind="ExternalInput").ap()
    woP = nc.dram_tensor("woP", [P, HPC, D], bf16, kind="ExternalInput").ap()
    y = nc.dram_tensor("y", [TOK, D], bf16, kind="ExternalOutput").ap()

    inv_sqrt_hd = 1.0 / math.sqrt(HD)

    with tile.TileContext(nc) as tc:
        with (
            tc.tile_pool(name="consts", bufs=1) as consts,
            tc.tile_pool(name="wpool", bufs=1) as wpool,
            tc.tile_pool(name="qkv", bufs=1) as qkv,
            tc.tile_pool(name="xp", bufs=4) as xp,
            tc.tile_pool(name="csp", bufs=2) as csp,
            tc.tile_pool(name="ropep", bufs=1) as ropep,
            tc.tile_pool(name="ptp", bufs=8) as ptp,
            tc.tile_pool(name="pap", bufs=2) as pap,
            tc.tile_pool(name="rrp", bufs=2) as rrp,
            tc.tile_pool(name="bcp", bufs=2) as bcp,
            tc.tile_pool(name="onp", bufs=3) as onp,
            tc.tile_pool(name="ysp", bufs=3) as ysp,
            tc.tile_pool(name="ps", bufs=8, space="PSUM") as ps,
        ):
            # ---- constants ----
            # single [128,128] triangular causal mask: keep q_local >= key_local
            mask128 = consts.tile([P, P], bf16, tag="mask128")
            nc.gpsimd.memset(mask128[:], 1.0)
            nc.gpsimd.affine_select(
                out=mask128[:], in_=mask128[:], compare_op=mybir.AluOpType.is_ge,
                fill=0.0, base=0, channel_multiplier=-1, pattern=[[1, P]],
            )
            ones_col = consts.tile([P, 1], bf16, tag="ones_col")
            nc.gpsimd.memset(ones_col[:], 1.0)

            # ---- resident weights ----
            wq_t = wpool.tile([P, CCHUNKS, DC], bf16, tag="wq")
            wk_t = wpool.tile([P, CCHUNKS, DC], bf16, tag="wk")
            wv_t = wpool.tile([P, CCHUNKS, DC], bf16, tag="wv")
            wo_t = wpool.tile([P, HPC, D], bf16, tag="wo")

            # chunk ranges per x-pair slot: small first part so chunk-0
            # matmuls are not stuck behind 768KB of weight DMA at cold start
            W_PARTS = [(0, 2), (2, 6), (6, 10), (10, 14), (14, 16)]

            def emit_w_part(j):
                csl = slice(*W_PARTS[j])
                for wt, wdram in ((wq_t, wqP), (wk_t, wkP), (wv_t, wvP)):
                    nc.scalar.dma_start(wt[:, csl, :], wdram[:, csl, :])

            # ---- resident activations ----
            qT_t = qkv.tile([P, HPC, TOK], bf16, tag="qT")  # [head_dim, h, tok]
            kT_t = qkv.tile([P, HPC, TOK], bf16, tag="kT")
            v_t = qkv.tile([P, TOK // P, DC], bf16, tag="v")  # [tok%128, tokblk, d]

            # ---- token-tile body: q/k/v projections + RoPE ----
            def emit_tile(tt, hook=None):
                tsl = slice(tt * 512, (tt + 1) * 512)
                cos_t = csp.tile([P, 512], bf16, tag="cos")
                nc.scalar.dma_start(cos_t[:], cosT[:, tsl])
                sin_t = csp.tile([P, 512], bf16, tag="sin")
                nc.scalar.dma_start(sin_t[:], sinT[:, tsl])

                pq = [ps.tile([P, 512], f32, tag="ps", name=f"pq{i}") for i in range(HPC)]
                pk = [ps.tile([P, 512], f32, tag="ps", name=f"pk{i}") for i in range(HPC)]
                # two banks hold all four v accumulators ([t128, 256] pairs packed
                # side by side).  Only the first half's c==0 matmul uses start=True
                # (clears the whole bank); the second half's first matmul then
                # overwrites its still-clean elements via has_written bits.
                pv = [ps.tile([P, 512], f32, tag="ps", name=f"pv{i}") for i in range(2)]

                for cp in range(CPAIRS):
                    if cp == 2 and hook is not None:
                        hook()
                    xt = xp.tile([P, 1024], bf16, tag="x")
                    # partition-split halves: two hardware DMA queues per pair
                    nc.sync.dma_start(xt[0:64, :], xTt[tt, cp, 0:64, :])
                    nc.sync.dma_start(xt[64:128, :], xTt[tt, cp, 64:128, :])
                    if tt == 0 and cp < len(W_PARTS):
                        emit_w_part(cp)
                    if tt == 1 and cp < 4:
                        # wo in 4 parts (2KB lines); first needed by the
                        # yproj filler dripped into attention unit (0,1)
                        h, nh = divmod(cp, 2)
                        nc.scalar.dma_start(
                            wo_t[:, h, nh * 1024:(nh + 1) * 1024],
                            woP[:, h, nh * 1024:(nh + 1) * 1024])
                    for ci in range(2):
                        c = 2 * cp + ci
                        xtr = xt[:, ci * 512:(ci + 1) * 512]
                        st, sp = (c == 0), (c == CCHUNKS - 1)
                        for h in range(HPC):
                            dsl = slice(h * HD, (h + 1) * HD)
                            nc.tensor.matmul(pq[h][:], wq_t[:, c, dsl], xtr,
                                             start=st, stop=sp)
                            nc.tensor.matmul(pk[h][:], wk_t[:, c, dsl], xtr,
                                             start=st, stop=sp)
                        for s4 in range(4):
                            half = s4 % 2
                            nc.tensor.matmul(pv[s4 // 2][:, half * DC:(half + 1) * DC],
                                             xt[:, ci * 512 + s4 * P:ci * 512 + (s4 + 1) * P],
                                             wv_t[:, c, :],
                                             start=st and half == 0, stop=sp,
                                             skip_group_check=half == 1)

                # Evacuate PSUM: q copies (ACT) feed the q RoPE immediately —
                # the next attention unit's first scores wait on it.  k copies
                # (DVE) and k RoPE after the q chain; v copies on ACT.
                for h in range(HPC):
                    nc.scalar.copy(qT_t[:, h, tsl], pq[h][:])

                def rope(dst_t, h):
                    dst = dst_t[:, h, tsl]
                    rot = ropep.tile([P, 512], bf16, tag="rot")
                    nc.vector.tensor_scalar_mul(rot[0:64, :], dst[64:128, :], -1.0)
                    nc.vector.tensor_copy(rot[64:128, :], dst[0:64, :])
                    nc.vector.tensor_mul(out=rot[:], in0=rot[:], in1=sin_t[:])
                    nc.vector.tensor_mul(out=dst, in0=dst, in1=cos_t[:])
                    nc.vector.tensor_add(out=dst, in0=dst, in1=rot[:])

                rope(qT_t, 0)
                nc.vector.tensor_copy(kT_t[:, 0, tsl], pk[0][:])
                rope(kT_t, 0)
                rope(qT_t, 1)
                nc.vector.tensor_copy(kT_t[:, 1, tsl], pk[1][:])
                rope(kT_t, 1)
                for s4 in range(4):
                    half = s4 % 2
                    nc.scalar.copy(v_t[:, tt * 4 + s4, :],
                                   pv[s4 // 2][:, half * DC:(half + 1) * DC])

            # ---- output projection of a finished unit, as a list of 16 small
            # steps (one PSUM bank: 2 matmuls + evacuation) for dripping into
            # the next unit's attention as PE filler ----
            def make_yproj_steps(onorm, b, qt, all_dve):
                ystages = {}

                def step(s4, dout):
                    if dout == 0:
                        ystages[s4] = ysp.tile([P, D], bf16, tag="ystage",
                                               name="ystage")
                    ystage = ystages[s4]
                    py = ps.tile([P, 512], f32, tag="ps", name="py")
                    for h in range(HPC):
                        nc.tensor.matmul(
                            py[:],
                            onorm[:, h, s4 * P:(s4 + 1) * P],
                            wo_t[:, h, dout * 512:(dout + 1) * 512],
                            start=(h == 0), stop=(h == HPC - 1))
                    # PSUM evacuation: while attention runs, ACT must stay
                    # a pure exp stream (a copy between exps stalls the AV
                    # chain), so filler copies go to DVE; the epilogue drain
                    # (no exps pending) alternates.
                    if not all_dve and (s4 * 4 + dout) % 2 == 0:
                        nc.scalar.copy(ystage[:, dout * 512:(dout + 1) * 512], py[:])
                    else:
                        nc.vector.tensor_copy(ystage[:, dout * 512:(dout + 1) * 512], py[:])
                    if dout == 3:
                        r0 = b * T + qt * 512 + s4 * P
                        nc.sync.dma_start(y[r0:r0 + P, :], ystage[:])

                return [(step, s4, dout) for s4 in range(4) for dout in range(4)]

            pending = []
            norm_queue = []

            def finish_norm():
                # row-sum matmuls + normalization of the previous unit,
                # emitted a few chunks into the next tile's burst: by then
                # the unit's DVE tail (last pacc adds) has long drained, so
                # the pr matmuls slot into the burst without a PE wait.
                if not norm_queue:
                    return
                paccs, onorm, b, qt = norm_queue.pop(0)
                for h in range(HPC):
                    pr = ps.tile([P, 512], f32, tag="ps", name="pr")
                    nc.tensor.matmul(pr[0:1, :], ones_col[:], paccs[h][:],
                                     start=True, stop=True)
                    rr = rrp.tile([1, 512], f32, tag="rr")
                    nc.vector.reciprocal_approx_fast(rr[:], pr[0:1, :])
                    bc = bcp.tile([P, 512], f32, tag="bc")
                    nc.gpsimd.partition_broadcast(bc[:], rr[:])
                    nc.vector.tensor_mul(out=onorm[:, h, :],
                                         in0=onorm[:, h, :], in1=bc[:])

            # ---- attention for one (batch, query-tile) unit.  Filler is a
            # global FIFO of yproj steps: short units (PE surplus) under-
            # consume and carry steps forward into the ACT-exp-bound long
            # units, which need the extra PE work per kt slot. ----
            FILL_RATE = {1: 0.7, 2: 0.9}

            def emit_attn(b, qt, fill_q):
                nkt = KT_PER_Q * (qt + 1)
                fstate = [0.0]

                def run_steps(n):
                    while n > 0 and fill_q:
                        f, s4, dout = fill_q.pop(0)
                        f(s4, dout)
                        n -= 1

                # seam: PE filler while DVE finishes the fresh tile's RoPE.
                # qt==0 units attend ONLY the fresh tile (every score waits on
                # its k RoPE), so they take a large upfront block.
                run_steps(14 if qt == 0 else SEAM_STEPS)
                if qt == 3:
                    per_slot = len(fill_q) / (HPC * nkt)
                else:
                    per_slot = FILL_RATE.get(qt, 0.0)

                def drip():
                    fstate[0] += per_slot
                    n = int(fstate[0])
                    if n:
                        fstate[0] -= n
                        run_steps(n)

                q0 = b * T + qt * 512
                onorm = onp.tile([P, HPC, 512], bf16, tag="onorm")
                paccs = []
                for h in range(HPC):
                    po = ps.tile([P, 512], f32, tag="ps")
                    pacc = pap.tile([P, 512], bf16, tag="pacc")

                    def emit_score(kt, b=b, qt=qt, h=h, q0=q0):
                        # diagonal-crossing tiles only need columns q >= kt*128
                        off = max(0, (kt - KT_PER_Q * qt) * P)
                        ksl = slice(b * T + kt * P, b * T + (kt + 1) * P)
                        pscore = ps.tile([P, 512], f32, tag="ps", name="pscore")
                        nc.tensor.matmul(pscore[:, off:], kT_t[:, h, ksl],
                                         qT_t[:, h, q0 + off:q0 + 512],
                                         start=True, stop=True)
                        ptile = ptp.tile([P, 512], bf16, tag="pt", name="ptile")
                        nc.scalar.activation(ptile[:, off:], pscore[:, off:],
                                             mybir.ActivationFunctionType.Exp,
                                             scale=inv_sqrt_hd)
                        if kt >= KT_PER_Q * qt:
                            # triangular mask on the single diagonal block
                            nc.vector.tensor_mul(out=ptile[:, off:off + P],
                                                 in0=ptile[:, off:off + P],
                                                 in1=mask128[:])
                        return ptile, off

                    # kt loop pipelined by three: scores run well ahead of
                    # the exp-gated AV matmuls, with filler between, so the
                    # PE has wait-free work during exp and the exp chain has
                    # elasticity against jitter (ACT is ~saturated here).
                    ptiles = {j: emit_score(j) for j in range(min(3, nkt))}
                    for kt in range(nkt):
                        if kt + 3 < nkt:
                            ptiles[kt + 3] = emit_score(kt + 3)
                        drip()
                        ptile, off = ptiles.pop(kt)
                        st, sp = (kt == 0), (kt == nkt - 1)
                        nc.tensor.matmul(po[:, off:], v_t[:, b * (T // P) + kt,
                                                          h * HD:(h + 1) * HD],
                                         ptile[:, off:], start=st, stop=sp,
                                         skip_group_check=off > 0)
                        # row-sum accumulation on DVE (all-bf16 2x mode)
                        if kt == 0:
                            nc.vector.tensor_copy(pacc[:], ptile[:])
                        else:
                            nc.vector.tensor_add(out=pacc[:, off:],
                                                 in0=pacc[:, off:],
                                                 in1=ptile[:, off:])
                    # copy o out of PSUM right away (frees the bank); the
                    # row-sum matmul + normalization are deferred into the
                    # NEXT tile's burst so the PE never waits on the last
                    # pacc add (DVE) — a wait there would also stall the next
                    # tile queued behind it.
                    if qt >= 2:
                        nc.vector.tensor_copy(onorm[:, h, :], po[:])
                    else:
                        nc.scalar.copy(onorm[:, h, :], po[:])
                    paccs.append(pacc)

                norm_queue.append((paccs, onorm, b, qt))
                pending.append((onorm, b, qt))

            # ---- schedule: iteration tt = dense tile burst, then attention
            # unit (b,qt)=divmod(tt,QT) with previous unit's yproj dripped in.
            # Tile-before-its-unit keeps the ACT queue flowing (tile PSUM
            # copies run at burst end, right before the unit's exps); the
            # fresh-RoPE wait of qt==0 units is covered by taking their
            # entire filler upfront. ----
            fill_q = []
            for tt in range(TT):
                b, qt = divmod(tt, QT)
                emit_tile(tt, hook=finish_norm)
                if pending:
                    fill_q.extend(make_yproj_steps(*pending.pop(0),
                                                   all_dve=True))
                emit_attn(b, qt, fill_q)
            finish_norm()
            for f, s4, dout in fill_q:
                f(s4, dout)
            for args in pending:
                for f, s4, dout in make_yproj_steps(*args, all_dve=False):
                    f(s4, dout)

    nc.compile()
    return nc


def get_nc():
    if "nc" not in _CACHE:
        _CACHE["nc"] = _build_nc()
    return _CACHE["nc"]


def make_in_maps(x, cos, sin, wq, wk, wv, wo):
    bf16 = ml_dtypes.bfloat16
    xT = np.asarray(x, dtype=np.float32).reshape(TOK, D).T  # [D, TOK]
    # [D, TOK] -> [cp, ci(2), 128, tt, 512] -> [tt, cp, 128, ci, 512]
    xTt = np.ascontiguousarray(
        xT.reshape(CPAIRS, 2, P, TT, 512).transpose(3, 0, 2, 1, 4)
    ).reshape(TT, CPAIRS, P, 1024).astype(bf16)
    cosT = np.ascontiguousarray(
        np.asarray(cos, dtype=np.float32).reshape(TOK, HD).T).astype(bf16)
    sinT = np.ascontiguousarray(
        np.asarray(sin, dtype=np.float32).reshape(TOK, HD).T).astype(bf16)
    wq = np.asarray(wq, dtype=np.float32)
    wk = np.asarray(wk, dtype=np.float32)
    wv = np.asarray(wv, dtype=np.float32)
    wo = np.asarray(wo, dtype=np.float32)

    def packw(w, dsl):
        # w[dsl, :].T is [D, DC] with row d = co*128 + ci; SBUF wants [ci, co, dc]
        return np.ascontiguousarray(
            w[dsl, :].T.reshape(CCHUNKS, P, DC).transpose(1, 0, 2)).astype(bf16)

    in_maps = []
    for c in range(NCORES):
        dsl = slice(c * DC, (c + 1) * DC)
        # wo[:, dsl].T is [DC, D] with row k = ko*128 + ki; SBUF wants [ki, ko, n]
        woPm = np.ascontiguousarray(
            wo[:, dsl].T.reshape(HPC, P, D).transpose(1, 0, 2)).astype(bf16)
        in_maps.append({
            "xTt": xTt,
            "cosT": cosT,
            "sinT": sinT,
            "wqP": packw(wq, dsl),
            "wkP": packw(wk, dsl),
            "wvP": packw(wv, dsl),
            "woP": woPm,
        })
    return in_maps


def kernel(x, cos, sin, wq, wk, wv, wo):
    from concourse.bass_utils import run_bass_kernel_spmd

    nc = get_nc()
    in_maps = make_in_maps(x, cos, sin, wq, wk, wv, wo)
    res = run_bass_kernel_spmd(nc, in_maps, list(range(NCORES)))
    out = np.zeros((TOK, D), dtype=np.float32)
    for m in res.results:
        out += m["y"].astype(np.float32)
    return out.reshape(B, T, D)
